# revision 8
# baseline (speedup 1.0000x reference)
"""Trainium2 Bass kernel for nn_AGTLayer (GAT-style additive-attention transformer layer).

Key algebraic fact exploited: softmax over j of (sl[i] + sr[j]) is independent
of sl (constant shift along the softmax axis), so the N x N attention matrix
collapses to a single weight vector per (batch, head):
    p[b,h,i,:] = softmax_j(sr[b,h,:])      (same for every query i)
    ctx[b,h,i,:] = sum_j p[b,h,j] fr[b,h,j,:]   (one vector per (b,h))
Hence fl / Wl / att_l never influence the output, and the layer reduces to:
    fr = h @ Wr.T
    sr[b,h,j] = leaky(fr[b,j,h*128:(h+1)*128]) . att_r
    w = exp(sr)  (values are O(1); no max subtraction needed)
    c[b,h,:] = (sum_j w[j] fr[b,j,head]) / sum_j w[j]
    fh[b,:] = concat_h(c[b,h,:]) @ Wf.T
    out = LayerNorm(h + fh[:,None,:]) * gamma + beta

Sharding: flatten (B,N) -> 8192 rows, 1024 rows per core (cores 2b, 2b+1 hold
batch b). Each core computes fr for its rows, partial softmax sums, then a
tiny pairwise AllReduce (4.1KB) combines the halves; each core redundantly
computes fh for its batch and applies the LayerNorm epilogue to its rows.

Hardware notes baked in:
 - PSUM `start=True` clears the whole bank, so every accumulation group gets
   a bank to itself (cpart is computed as w.T @ fr -> [8, 1024] with one
   group per 512-wide bank, not per-head matvecs).
 - ACT LUT reloads on function switch cost ~1.3us, so Exp is batched into a
   single call after the main loop instead of per-tile.
"""

import numpy as np
import ml_dtypes
from contextlib import ExitStack

import concourse.bass as bass
import concourse.mybir as mybir
import concourse.tile as tile
from concourse import bacc
from concourse.bass_utils import run_bass_kernel_spmd

AF = mybir.ActivationFunctionType
ALU = mybir.AluOpType
F32 = mybir.dt.float32
BF16 = mybir.dt.bfloat16

B, N, D, H, HD = 4, 2048, 1024, 8, 128
NCORES = 8
ROWS = (B * N) // NCORES  # 1024 rows per core
P = 128                   # partitions
KT = D // P               # 8 k-tiles
MT = ROWS // P            # 8 row-tiles per core
NB = 512                  # psum bank free-dim (f32)
LN_EPS = 1e-5


def _bcast_ap(ap, parts, free):
    """Partition-broadcast view of a [1, free] DRAM AP."""
    return bass.AP(tensor=ap.tensor, offset=ap.offset, ap=[[0, parts], [1, free]])


def _build(apply_gb: bool):
    nc = bacc.Bacc(
        "TRN2",
        target_bir_lowering=False,
        debug=False,
        enable_asserts=False,
        num_devices=NCORES,
    )

    hT = nc.dram_tensor("hT", [D, ROWS], BF16, kind="ExternalInput")
    hF = nc.dram_tensor("hF", [ROWS, D], F32, kind="ExternalInput")
    WrT = nc.dram_tensor("WrT", [D, D], BF16, kind="ExternalInput")
    WfT = nc.dram_tensor("WfT", [D, D], BF16, kind="ExternalInput")
    attr = nc.dram_tensor("attr", [1, D], BF16, kind="ExternalInput")
    out = nc.dram_tensor("out", [ROWS, D], F32, kind="ExternalOutput")
    if apply_gb:
        gam = nc.dram_tensor("gam", [1, D], F32, kind="ExternalInput")
        bet = nc.dram_tensor("bet", [1, D], F32, kind="ExternalInput")

    with tile.TileContext(nc) as tc, ExitStack() as ctx:
        const = ctx.enter_context(tc.tile_pool(name="const", bufs=1))
        work = ctx.enter_context(tc.tile_pool(name="work", bufs=3))
        ep = ctx.enter_context(tc.tile_pool(name="ep", bufs=3))
        eps_p = ctx.enter_context(tc.tile_pool(name="eps", bufs=4))
        frp = ctx.enter_context(tc.tile_pool(name="frp", bufs=2, space="PSUM"))
        accp = ctx.enter_context(tc.tile_pool(name="accp", bufs=1, space="PSUM"))
        sp = ctx.enter_context(tc.tile_pool(name="sp", bufs=1, space="PSUM"))
        dram = ctx.enter_context(tc.tile_pool(name="dram", bufs=1, space="DRAM"))

        # ---- constants / persistent loads ----
        att_sb = const.tile([P, D], BF16, tag="att")
        nc.sync.dma_start(out=att_sb[:], in_=_bcast_ap(attr.ap(), P, D))
        ones_m = const.tile([P, 1], BF16, tag="ones_m")
        nc.vector.memset(ones_m[:], 1.0)
        ones1 = const.tile([1, P], F32, tag="ones1")
        nc.vector.memset(ones1[:], 1.0)
        eps_sb = const.tile([P, 1], F32, tag="eps")
        nc.vector.memset(eps_sb[:], LN_EPS)
        if apply_gb:
            gam_sb = const.tile([P, D], F32, tag="gam")
            nc.sync.dma_start(out=gam_sb[:], in_=_bcast_ap(gam.ap(), P, D))
            bet_sb = const.tile([P, D], F32, tag="bet")
            nc.sync.dma_start(out=bet_sb[:], in_=_bcast_ap(bet.ap(), P, D))

        ht_t, wr_t, wf_t, h_t = [], [], [], []
        for kt in range(KT):
            t = const.tile([P, ROWS], BF16, tag=f"ht{kt}")
            nc.sync.dma_start(out=t[:], in_=hT.ap()[kt * P:(kt + 1) * P, :])
            ht_t.append(t)
            t = const.tile([P, D], BF16, tag=f"wr{kt}")
            nc.sync.dma_start(out=t[:], in_=WrT.ap()[kt * P:(kt + 1) * P, :])
            wr_t.append(t)
        for kt in range(KT):
            t = const.tile([P, D], BF16, tag=f"wf{kt}")
            nc.sync.dma_start(out=t[:], in_=WfT.ap()[kt * P:(kt + 1) * P, :])
            wf_t.append(t)
        for mt in range(MT):
            t = const.tile([P, D], F32, tag=f"h{mt}")
            nc.sync.dma_start(out=t[:], in_=hF.ap()[mt * P:(mt + 1) * P, :])
            h_t.append(t)

        # ---- main loop: fr matmul + leaky/att/reduce into sr_all ----
        sr_all = const.tile([P, MT, H], F32, tag="sr_all")
        frb_t = []
        for mt in range(MT):
            fr_ps = frp.tile([P, D], F32, tag="fr")
            for kt in range(KT):
                lw = ht_t[kt][:, mt * P:(mt + 1) * P]
                for nh in range(2):
                    nc.tensor.matmul(
                        fr_ps[:, nh * NB:(nh + 1) * NB],
                        lhsT=lw,
                        rhs=wr_t[kt][:, nh * NB:(nh + 1) * NB],
                        start=(kt == 0),
                        stop=(kt == KT - 1),
                    )
            frb = const.tile([P, D], BF16, tag=f"frb{mt}")
            nc.scalar.activation(out=frb[:], in_=fr_ps[:], func=AF.Copy)
            lky = work.tile([P, D], BF16, tag="lky")
            nc.scalar.activation(out=lky[:], in_=fr_ps[:], func=AF.Lrelu,
                                 alpha=0.01)
            t2 = work.tile([P, D], BF16, tag="t2")
            nc.vector.tensor_tensor(out=t2[:], in0=lky[:], in1=att_sb[:],
                                    op=ALU.mult)
            nc.vector.tensor_reduce(
                out=sr_all[:, mt, :],
                in_=t2[:].rearrange("p (h hd) -> p h hd", h=H),
                axis=mybir.AxisListType.X,
                op=ALU.add,
            )
            frb_t.append(frb)

        # ---- batched exp, then cpart = w.T @ fr  (one group per PSUM bank) ----
        w_all = const.tile([P, MT, H], BF16, tag="w_all")
        nc.scalar.activation(out=w_all[:], in_=sr_all[:], func=AF.Exp)

        cs_ps = accp.tile([H, D], F32, tag="acc")
        s_ps = sp.tile([H, 1], F32, tag="s")
        for mt in range(MT):
            for nh in range(2):
                nc.tensor.matmul(
                    cs_ps[0:H, nh * NB:(nh + 1) * NB],
                    lhsT=w_all[:, mt, :],
                    rhs=frb_t[mt][:, nh * NB:(nh + 1) * NB],
                    start=(mt == 0),
                    stop=(mt == MT - 1),
                )
            nc.tensor.matmul(s_ps[0:H, 0:1], lhsT=w_all[:, mt, :],
                             rhs=ones_m[:], start=(mt == 0),
                             stop=(mt == MT - 1))

        # ---- pairwise AllReduce of (diag blocks of cpart, s) ----
        cs_sb = const.tile([H, D], F32, tag="cs_sb")
        nc.vector.tensor_copy(out=cs_sb[:], in_=cs_ps[:])
        s_sb = const.tile([H, 1], F32, tag="s_sb")
        nc.vector.tensor_copy(out=s_sb[:], in_=s_ps[:])

        cc_in = dram.tile([1, D + H], F32, tag="ccin")
        cc_out = dram.tile([1, D + H], F32, tag="ccout")
        for hh in range(H):
            nc.gpsimd.dma_start(out=cc_in[0:1, hh * HD:(hh + 1) * HD],
                                in_=cs_sb[hh:hh + 1, hh * HD:(hh + 1) * HD])
        nc.gpsimd.dma_start(out=cc_in[0:1, D:D + H], in_=s_sb[:])
        nc.gpsimd.collective_compute(
            "AllReduce",
            ALU.add,
            replica_groups=[[0, 1], [2, 3], [4, 5], [6, 7]],
            ins=[cc_in[:].opt()],
            outs=[cc_out[:].opt()],
        )
        csum = const.tile([P, H], F32, tag="csum")
        cbase = cc_out[0:1, 0:1]
        nc.gpsimd.dma_start(
            out=csum[:],
            in_=bass.AP(tensor=cbase.tensor, offset=cbase.offset,
                        ap=[[1, P], [P, H]]),
        )
        ssum = const.tile([1, H], F32, tag="ssum")
        nc.gpsimd.dma_start(out=ssum[:], in_=cc_out[0:1, D:D + H])

        # c = cpart / s  (per head); broadcast 1/s across partitions via PE
        rs = const.tile([1, H], F32, tag="rs")
        nc.vector.reciprocal(out=rs[:], in_=ssum[:])
        rsb_ps = accp.tile([P, H], F32, tag="acc")  # reuses cs_ps slot
        nc.tensor.matmul(rsb_ps[:], lhsT=ones1[:], rhs=rs[:], start=True,
                         stop=True)
        cn = const.tile([P, H], BF16, tag="cn")
        nc.vector.tensor_tensor(out=cn[:], in0=csum[:], in1=rsb_ps[:],
                                op=ALU.mult)

        # fh = c @ Wf.T  (matvec over k-tiles; column h of cn is k-tile h)
        fh_ps = accp.tile([1, D], F32, tag="acc")  # slot free after rsb is read
        for kt in range(KT):
            for nh in range(2):
                nc.tensor.matmul(
                    fh_ps[0:1, nh * NB:(nh + 1) * NB],
                    lhsT=cn[:, kt:kt + 1],
                    rhs=wf_t[kt][:, nh * NB:(nh + 1) * NB],
                    start=(kt == 0),
                    stop=(kt == KT - 1),
                )
        fh_dram = dram.tile([1, D], F32, tag="fhd")
        fh_sb = const.tile([1, D], F32, tag="fh_sb")
        nc.vector.tensor_copy(out=fh_sb[:], in_=fh_ps[:])
        nc.gpsimd.dma_start(out=fh_dram[:], in_=fh_sb[:])
        fhb = const.tile([P, D], F32, tag="fhb")
        fd = fh_dram[0:1, :]
        nc.gpsimd.dma_start(
            out=fhb[:],
            in_=bass.AP(tensor=fd.tensor, offset=fd.offset, ap=[[0, P], [1, D]]),
        )

        # ---- epilogue: y = h + fh, LayerNorm over d, write out ----
        for mt in range(MT):
            y = ep.tile([P, D], F32, tag="y")
            nc.gpsimd.tensor_tensor(out=y[:], in0=h_t[mt][:], in1=fhb[:],
                                    op=ALU.add)
            st = eps_p.tile([P, 2, 6], F32, tag="st")
            nc.vector.bn_stats(out=st[:, 0, :], in_=y[:, 0:NB])
            nc.vector.bn_stats(out=st[:, 1, :], in_=y[:, NB:D])
            mv = eps_p.tile([P, 2], F32, tag="mv")
            nc.vector.bn_aggr(out=mv[:], in_=st[:])
            sd = eps_p.tile([P, 1], F32, tag="sd")
            nc.scalar.activation(out=sd[:], in_=mv[:, 1:2], func=AF.Sqrt,
                                 bias=eps_sb[:])
            rstd = eps_p.tile([P, 1], F32, tag="rstd")
            nc.vector.reciprocal(out=rstd[:], in_=sd[:])
            o = ep.tile([P, D], F32, tag="o")
            nc.vector.tensor_scalar(
                out=o[:], in0=y[:],
                scalar1=mv[:, 0:1], scalar2=rstd[:],
                op0=ALU.subtract, op1=ALU.mult,
            )
            if apply_gb:
                nc.gpsimd.tensor_tensor(out=o[:], in0=o[:], in1=gam_sb[:],
                                        op=ALU.mult)
                nc.gpsimd.tensor_tensor(out=o[:], in0=o[:], in1=bet_sb[:],
                                        op=ALU.add)
            nc.sync.dma_start(out=out.ap()[mt * P:(mt + 1) * P, :], in_=o[:])

    nc.compile()
    return nc


_NC_CACHE = {}


def _get_nc(apply_gb: bool):
    if apply_gb not in _NC_CACHE:
        _NC_CACHE[apply_gb] = _build(apply_gb)
    return _NC_CACHE[apply_gb]


def _make_in_maps(h, Wr, att_r, Wf, ln_gamma, ln_beta, apply_gb):
    hf = np.ascontiguousarray(np.asarray(h, np.float32).reshape(B * N, D))
    WrT = np.ascontiguousarray(np.asarray(Wr, np.float32).T).astype(
        ml_dtypes.bfloat16)
    WfT = np.ascontiguousarray(np.asarray(Wf, np.float32).T).astype(
        ml_dtypes.bfloat16)
    at = np.tile(np.asarray(att_r, np.float32).reshape(1, HD), (1, H)).astype(
        ml_dtypes.bfloat16)
    in_maps = []
    for i in range(NCORES):
        sh = hf[i * ROWS:(i + 1) * ROWS]
        m = {
            "hT": np.ascontiguousarray(sh.T).astype(ml_dtypes.bfloat16),
            "hF": sh,
            "WrT": WrT,
            "WfT": WfT,
            "attr": at,
        }
        if apply_gb:
            m["gam"] = np.asarray(ln_gamma, np.float32).reshape(1, D)
            m["bet"] = np.asarray(ln_beta, np.float32).reshape(1, D)
        in_maps.append(m)
    return in_maps


def _run(h, Wl, Wr, att_l, att_r, Wf, ln_gamma, ln_beta, trace=False):
    g = np.asarray(ln_gamma, np.float32)
    b = np.asarray(ln_beta, np.float32)
    apply_gb = not (np.all(g == 1.0) and np.all(b == 0.0))
    nc = _get_nc(apply_gb)
    in_maps = _make_in_maps(h, Wr, att_r, Wf, ln_gamma, ln_beta, apply_gb)
    res = run_bass_kernel_spmd(nc, in_maps, core_ids=list(range(NCORES)),
                               trace=trace)
    outs = [res.results[i]["out"] for i in range(NCORES)]
    full = np.concatenate(outs, axis=0).reshape(B, N, D).astype(np.float32)
    return full, res


def kernel(**inputs):
    out, _ = _run(**inputs)
    return out
